# revision 1
# baseline (speedup 1.0000x reference)
"""Bass/Trainium2 kernel for nn_Attention_84688165142614 (additive attention).

Computes, for full inputs (B=32, S=2048, EH=512, DH=512):
    enc    = enc_output.transpose(1, 0, 2)                  # [B, S, 2EH]
    energy = tanh(enc @ w_enc + (h @ w_dec) + attn_b)       # [B, S, DH]
    att    = energy @ v_w                                   # [B, S]
    att    = where(mask == 0, -1e10, att)
    out    = softmax(att, axis=1)

Strategy: data-parallel over batch across 8 NeuronCores (4 batches/core),
plus mask-sparsity compaction. The mask is ~50% zeros and masked positions
produce exactly 0 in the reference output (exp(-1e10) underflows in f32),
so the host keeps only unmasked source positions per batch (gather),
pads each batch to a multiple of 128 columns, transposes the kept enc
columns feature-major and pre-casts to bf16. Batches are assigned to
(core, slot) by sorted compacted width, so the SPMD per-slot tile counts
are the max over cores of the k-th widest batch — for a p=0.5 mask this
drops one padded 128-column tile from most slots. The device computes
energies/logits/softmax only for the compacted columns (pads are killed
with a -1e10 additive mask row), and the host scatters the compacted
probabilities back into a zero [B, S] output.

Device structure: the big matmul runs in bf16 with fp32 PSUM accumulation.
The enc DRAM shard is laid out group-major — exactly the order the PE
consumes it (per slot, per PSUM group of 4 s-tiles, ec-major inside) —
so each (slot, group) is one contiguous DMA into its own SBUF tile, all
on the single sync HWDGE ring in consumption order (SDMA round-robins
across rings at packet granularity, so a single FIFO ring is the only
way to prioritize; it still spreads across all 16 SDMA engines). Warmup
matmuls on memset data keep the PE busy (and the HAM clock un-throttled)
during the initial fill. tanh output and v are bf16. Softmax skips
max-subtraction (logits are bounded by sum|v| ~ 8); all logits are
shifted by -8*ln2 (softmax is shift-invariant) so exp sums fit fp16,
letting the partition-sum broadcast matmul run 1-pass fp16 instead of
2-pass fp32. The last slot folds the dec add into the matmul
accumulation so the vector backlog drains before the kernel tail.
"""

import numpy as np
from contextlib import ExitStack

import concourse.bass as bass
import concourse.tile as tile
from concourse import bacc, mybir
from concourse.bass_utils import run_bass_kernel_spmd

# Problem shape (hardcoded; kernel.py must be self-contained).
B, S, E2, DH = 32, 2048, 1024, 512
N_CORES = 8
BC = B // N_CORES        # batches per core = 4
P = 128                  # SBUF partitions
EC = E2 // P             # enc-feature chunks = 8
D = DH                   # 512
KC = DH // P             # dec-feature chunks = 4

f32 = mybir.dt.float32
bf16 = mybir.dt.bfloat16
fp16 = mybir.dt.float16
AF = mybir.ActivationFunctionType
ALU = mybir.AluOpType

NEG_BIG = -1.0e10
SHIFT = 8.0 * 0.6931471805599453  # logit shift so exp sums fit fp16

_NC_CACHE = {}


def _group_sizes(nt):
    sizes = [4] * (nt // 4)
    if nt % 4:
        sizes.append(nt % 4)
    return sizes


def _slot_group_sizes(widths, b):
    """PSUM-group sizes for slot b. The LAST slot ends with a 1-tile group,
    so only one tanh+amr drains after the kernel's final matmul instead of
    a whole 4-tile group's worth."""
    w = widths[b]
    if b == len(widths) - 1 and w > 1:
        return _group_sizes(w - 1) + [1]
    return _group_sizes(w)


def _emit(ctx, tc, nc, widths, enc_t, hwdec, madd_in, w_enc, sel_in, bv_in, out):
    nslots = len(widths)
    toff = [sum(widths[:i]) for i in range(nslots)]        # tile offset per slot
    ntot = sum(widths)
    slot_sizes = [_slot_group_sizes(widths, b) for b in range(nslots)]
    # column offset (in the EC-major enc layout) of (slot, group)
    coff = {}
    c = 0
    for b in range(nslots):
        for sg, gsz in enumerate(slot_sizes[b]):
            coff[(b, sg)] = c
            c += EC * gsz * P

    const = ctx.enter_context(tc.tile_pool(name="const", bufs=1))
    spsum = ctx.enter_context(tc.tile_pool(name="spsum", bufs=1, space="PSUM"))
    mpsum = ctx.enter_context(tc.tile_pool(name="mpsum", bufs=7, space="PSUM"))
    ngrp = sum(len(s) for s in slot_sizes)
    encp = ctx.enter_context(tc.tile_pool(name="encp", bufs=ngrp))
    tmpp = ctx.enter_context(tc.tile_pool(name="tmpp", bufs=3))
    thp = ctx.enter_context(tc.tile_pool(name="thp", bufs=5))
    epip = ctx.enter_context(tc.tile_pool(name="epip", bufs=12))

    # ---- warmup source tiles (no DMA deps): keep the PE busy during fill ----
    ones16 = const.tile([P, P], fp16)
    nc.vector.memset(ones16[:], 1.0)
    wsrc = const.tile([P, D], fp16)
    nc.vector.memset(wsrc[:], 0.001)
    ones_row = const.tile([1, P], bf16)
    nc.vector.memset(ones_row[:], 1.0)

    # ---- DMA: single sync ring in exact consumption order ----
    gtiles = {}
    for b in range(nslots):
        for sg, gsz in enumerate(slot_sizes[b]):
            gtiles[(b, sg)] = encp.tile(
                [P, EC * gsz * P], bf16, tag="enc", name=f"enc_{b}_{sg}"
            )

    wq = const.tile([P, EC * D], bf16)
    hwdec_sb = const.tile([P, KC * BC + KC * D], bf16)
    madd_sb = const.tile([P, ntot], f32)
    sel_sb = const.tile([BC, BC * P], bf16)
    bv_sb = const.tile([1, 2 * D], bf16)

    g00 = gtiles[(0, 0)]
    Wg0 = slot_sizes[0][0] * P
    # Opening transfers, all on the sync ring interleaved in consumption
    # order. Completion sems chain serially per ring (~2.4us apart), so few
    # coarse transfers beat many fine ones, and keeping the chain on one
    # ring is the most robust ordering across runs (scalar-ring sems race
    # behind the ACT table load).
    nc.sync.dma_start(out=wq[:, 0 : 2 * D], in_=w_enc[:, 0 : 2 * D])
    nc.sync.dma_start(out=g00[:, 0 : 4 * Wg0], in_=enc_t[:, 0 : 4 * Wg0])
    nc.sync.dma_start(out=wq[:, 2 * D : 4 * D], in_=w_enc[:, 2 * D : 4 * D])
    nc.sync.dma_start(out=g00[:, 4 * Wg0 : 8 * Wg0], in_=enc_t[:, 4 * Wg0 : 8 * Wg0])
    nc.sync.dma_start(out=wq[:, 4 * D : 8 * D], in_=w_enc[:, 4 * D : 8 * D])

    # small consts on the scalar ring after the wq chunks
    nc.scalar.dma_start(out=hwdec_sb[:], in_=hwdec[:])
    nc.scalar.dma_start(out=madd_sb[:], in_=madd_in[:])
    nc.scalar.dma_start(out=sel_sb[:], in_=sel_in[:])
    nc.scalar.dma_start(out=bv_sb[:], in_=bv_in[:])

    # remaining groups on the sync ring in consumption order
    rest = [(b, sg) for b in range(nslots) for sg in range(len(slot_sizes[b]))][1:]
    for b, sg in rest:
        gsz = slot_sizes[b][sg]
        lo = coff[(b, sg)]
        hi = lo + EC * gsz * P
        nc.sync.dma_start(out=gtiles[(b, sg)][:], in_=enc_t[:, lo:hi])

    # ---- PE warmup: 8 matmuls on memset data (~3.4us cold, HAM -> 8/8) ----
    wps = spsum.tile([P, D], f32, tag="sp", name="warm")
    for i in range(8):
        nc.tensor.matmul(wps[:], lhsT=ones16[:], rhs=wsrc[:], start=True, stop=True)

    # ---- dec[b, :] = h[slot b] @ w_dec + attn_b; broadcasts ----
    HW0 = KC * BC  # offset of w_dec columns inside hwdec_sb
    dec_ps = spsum.tile([BC, D], f32, tag="sp")
    for kc in range(KC):
        nc.tensor.matmul(
            dec_ps[:],
            lhsT=hwdec_sb[:, kc * BC : (kc + 1) * BC],
            rhs=hwdec_sb[:, HW0 + kc * D : HW0 + (kc + 1) * D],
            start=(kc == 0),
            stop=False,
        )
    nc.tensor.matmul(
        dec_ps[:], lhsT=ones_row[:, 0:BC], rhs=bv_sb[:, 0:D], start=False, stop=True
    )
    dec_rows = const.tile([BC, D], bf16)
    nc.vector.tensor_copy(dec_rows[:], dec_ps[:])

    dec_bc = const.tile([P, BC * D], f32)
    for b in range(nslots):
        ps = spsum.tile([P, D], f32, tag="sp", name=f"decb_{b}")
        nc.tensor.matmul(
            ps[:], lhsT=sel_sb[:, b * P : (b + 1) * P], rhs=dec_rows[:],
            start=True, stop=True,
        )
        nc.vector.tensor_copy(dec_bc[:, b * D : (b + 1) * D], ps[:])
    v_ps = spsum.tile([P, D], f32, tag="sp")
    nc.tensor.matmul(
        v_ps[:], lhsT=ones_row[:], rhs=bv_sb[:, D : 2 * D], start=True, stop=True
    )
    v_sb = const.tile([P, D], bf16)
    nc.vector.tensor_copy(v_sb[:], v_ps[:])

    # ---- main loop over slots ----
    for b in range(nslots):
        nt = widths[b]
        sizes = slot_sizes[b]
        starts = [sum(sizes[:i]) for i in range(len(sizes))]
        # last slot: dec add folded into the accumulation so tanh reads PSUM
        # and the vector backlog drains before the kernel tail
        dec_in_mm = b == nslots - 1
        att = epip.tile([P, nt], f32, tag="att", name=f"att_{b}")
        for sg, gsz in enumerate(sizes):
            gt = gtiles[(b, sg)]
            Wg = gsz * P
            psums = [
                mpsum.tile([P, D], f32, tag="mm", name=f"mm_{b}_{sg}_{j}")
                for j in range(gsz)
            ]
            for ec in range(EC):
                for j in range(gsz):
                    nc.tensor.matmul(
                        psums[j][:],
                        lhsT=gt[:, ec * Wg + j * P : ec * Wg + (j + 1) * P],
                        rhs=wq[:, ec * D : (ec + 1) * D],
                        start=(ec == 0),
                        stop=(ec == EC - 1) and not dec_in_mm,
                    )
            if dec_in_mm:
                for j in range(gsz):
                    nc.tensor.matmul(
                        psums[j][:],
                        lhsT=sel_sb[:, b * P : (b + 1) * P],
                        rhs=dec_rows[:],
                        start=False,
                        stop=True,
                    )
            for j in range(gsz):
                st = starts[sg] + j
                th = thp.tile([P, D], bf16, tag="th")
                if dec_in_mm:
                    nc.scalar.activation(th[:], psums[j][:], AF.Tanh)
                else:
                    t_sb = tmpp.tile([P, D], f32, tag="tmp")
                    nc.vector.tensor_add(
                        t_sb[:], psums[j][:], dec_bc[:, b * D : (b + 1) * D]
                    )
                    nc.scalar.activation(th[:], t_sb[:], AF.Tanh)
                scr = thp.tile([P, D], bf16, tag="scr")
                nc.vector.affine_mul_reduce(
                    out=scr[:],
                    accum_out=att[:, st : st + 1],
                    in0=th[:],
                    in1=v_sb[:],
                    scale=1.0,
                    bias=0.0,
                )

        # ---- epilogue: mask+shift, exp, fp16 partition-sum bcast, scale ----
        attm = epip.tile([P, nt], f32, tag="attm", name=f"attm_{b}")
        nc.vector.tensor_add(
            attm[:], att[:], madd_sb[:, toff[b] : toff[b] + nt]
        )
        expt = epip.tile([P, nt], f32, tag="expt", name=f"expt_{b}")
        nc.scalar.activation(expt[:], attm[:], AF.Exp)
        p16 = epip.tile([P, 1], fp16, tag="p16", name=f"p16_{b}")
        with nc.allow_low_precision(reason="fp16 softmax denominator rounding"):
            nc.vector.tensor_reduce(
                p16[:], expt[:], mybir.AxisListType.X, ALU.add
            )
        tot_ps = spsum.tile([P, 1], f32, tag="sp", name=f"tot_{b}")
        nc.tensor.matmul(
            tot_ps[:], lhsT=ones16[:], rhs=p16[:], start=True, stop=True
        )
        r_pp = epip.tile([P, 1], f32, tag="rpp", name=f"rpp_{b}")
        nc.vector.reciprocal(r_pp[:], tot_ps[:])
        out_sb = epip.tile([P, nt], f32, tag="outsb", name=f"osb_{b}")
        nc.vector.tensor_scalar_mul(out_sb[:], expt[:], r_pp[:])
        nc.scalar.dma_start(
            out=out[:, toff[b] : toff[b] + nt], in_=out_sb[:]
        )


def build_nc(widths):
    key = tuple(widths)
    if key in _NC_CACHE:
        return _NC_CACHE[key]
    ntot = sum(widths)
    nc = bacc.Bacc("TRN2", target_bir_lowering=False, debug=False)
    enc_t = nc.dram_tensor(
        "enc_t", [P, EC * P * ntot], bf16, kind="ExternalInput"
    ).ap()
    hwdec = nc.dram_tensor(
        "hwdec", [P, KC * BC + KC * D], bf16, kind="ExternalInput"
    ).ap()
    madd = nc.dram_tensor("madd", [P, ntot], f32, kind="ExternalInput").ap()
    w_enc = nc.dram_tensor("w_enc", [P, EC * D], bf16, kind="ExternalInput").ap()
    sel_in = nc.dram_tensor("sel_in", [BC, BC * P], bf16, kind="ExternalInput").ap()
    bv = nc.dram_tensor("bv", [1, 2 * D], bf16, kind="ExternalInput").ap()
    out = nc.dram_tensor("out", [P, ntot], f32, kind="ExternalOutput").ap()

    with tile.TileContext(nc) as tc:
        with ExitStack() as ctx:
            _emit(ctx, tc, nc, list(widths), enc_t, hwdec, madd, w_enc, sel_in,
                  bv, out)
    nc.compile()
    _NC_CACHE[key] = nc
    return nc


def plan_assignment(counts):
    """Sort batches by compacted tile count; rank k -> core k%8, slot k//8.
    Returns (assign[core][slot] = global batch, widths[slot])."""
    tiles = np.maximum(1, np.ceil(counts / P).astype(int))
    order = sorted(range(B), key=lambda gb: (-tiles[gb], -counts[gb], gb))
    assign = [[-1] * BC for _ in range(N_CORES)]
    widths = []
    for slot in range(BC):
        ranks = order[slot * N_CORES : (slot + 1) * N_CORES]
        for c, gb in enumerate(ranks):
            assign[c][slot] = gb
        widths.append(max(int(tiles[gb]) for gb in ranks))
    return assign, widths


def shard_inputs(inputs, assign, widths):
    import ml_dtypes

    h = np.asarray(inputs["h"], dtype=np.float32)
    enc = np.asarray(inputs["enc_output"], dtype=np.float32)
    mask = np.asarray(inputs["mask"], dtype=np.int32)
    attn_w = np.asarray(inputs["attn_w"], dtype=np.float32)
    attn_b = np.asarray(inputs["attn_b"], dtype=np.float32)
    v_w = np.asarray(inputs["v_w"], dtype=np.float32)

    ntot = sum(widths)
    toff = [sum(widths[:i]) for i in range(len(widths))]

    # w_dec [DH, D] -> [P, KC*D] with free index (kc, d)
    w_dec = np.ascontiguousarray(
        attn_w[:DH].reshape(KC, P, D).transpose(1, 0, 2).reshape(P, KC * D)
    )
    # w_enc [E2, D] -> [P, EC*D] with free index (ec, d), pre-cast to bf16
    w_enc = np.ascontiguousarray(
        attn_w[DH:].reshape(EC, P, D).transpose(1, 0, 2).reshape(P, EC * D)
    ).astype(ml_dtypes.bfloat16)

    sel_np = np.zeros((BC, BC * P), dtype=ml_dtypes.bfloat16)
    for b in range(BC):
        sel_np[b, b * P : (b + 1) * P] = 1.0
    bv = np.concatenate([attn_b, v_w]).reshape(1, 2 * D).astype(ml_dtypes.bfloat16)

    kept = [np.nonzero(mask[gb])[0] for gb in range(B)]

    in_maps = []
    for c in range(N_CORES):
        enc_c = np.zeros((P, EC * P * ntot), dtype=ml_dtypes.bfloat16)
        madd = np.full((P, ntot), NEG_BIG, dtype=np.float32)
        perm = assign[c]
        h_t = (
            h[perm]
            .T.reshape(KC, P, BC)
            .transpose(1, 0, 2)
            .reshape(P, KC * BC)
        )
        hwdec = np.concatenate([h_t, w_dec], axis=1).astype(ml_dtypes.bfloat16)
        col = 0
        for b in range(BC):
            gb = perm[b]
            W = widths[b] * P
            idx = kept[gb]
            n = len(idx)
            # kept enc columns, feature-major, padded: [EC, P, W]
            padded = np.zeros((EC, P, W), dtype=ml_dtypes.bfloat16)
            cols = enc[idx, gb, :].T.astype(ml_dtypes.bfloat16)
            padded[:, :, :n] = cols.reshape(EC, P, n)
            # group-major column order: per group sg, ec-major block
            off = 0
            for gsz in _slot_group_sizes(widths, b):
                blk = padded[:, :, off : off + gsz * P]      # [EC, P, Wg]
                w = EC * gsz * P
                enc_c[:, col : col + w] = blk.transpose(1, 0, 2).reshape(P, w)
                off += gsz * P
                col += w
            # compact additive mask: -shift for real columns, -1e10 for pads
            m = np.full(W, -SHIFT, dtype=np.float32)
            m[n:] = NEG_BIG
            madd[:, toff[b] : toff[b] + widths[b]] = m.reshape(widths[b], P).T
        in_maps.append(
            dict(
                enc_t=enc_c, hwdec=hwdec, madd=madd, w_enc=w_enc,
                sel_in=sel_np, bv=bv,
            )
        )
    return in_maps, kept


def run(inputs, trace=False):
    mask = np.asarray(inputs["mask"], dtype=np.int32)
    counts = mask.sum(axis=1)
    assign, widths = plan_assignment(counts)
    nc = build_nc(widths)
    in_maps, kept = shard_inputs(inputs, assign, widths)
    res = run_bass_kernel_spmd(nc, in_maps, list(range(N_CORES)), trace=trace)
    ntot = sum(widths)
    toff = [sum(widths[:i]) for i in range(len(widths))]
    out_full = np.zeros((B, S), dtype=np.float32)
    for c in range(N_CORES):
        vals = res.results[c]["out"].reshape(P, ntot)
        for b in range(BC):
            gb = assign[c][b]
            idx = kept[gb]
            w = widths[b]
            flat = vals[:, toff[b] : toff[b] + w].T.reshape(w * P)
            out_full[gb, idx] = flat[: len(idx)]
    return out_full, res


def kernel(**inputs) -> np.ndarray:
    out, _ = run(inputs, trace=False)
    return out



# revision 2
# speedup vs baseline: 1.0466x; 1.0466x over previous
"""Bass/Trainium2 kernel for nn_Attention_84688165142614 (additive attention).

Computes, for full inputs (B=32, S=2048, EH=512, DH=512):
    enc    = enc_output.transpose(1, 0, 2)                  # [B, S, 2EH]
    energy = tanh(enc @ w_enc + (h @ w_dec) + attn_b)       # [B, S, DH]
    att    = energy @ v_w                                   # [B, S]
    att    = where(mask == 0, -1e10, att)
    out    = softmax(att, axis=1)

Strategy: data-parallel over batch across 8 NeuronCores (4 batches/core),
plus mask-sparsity compaction. The mask is ~50% zeros and masked positions
produce exactly 0 in the reference output (exp(-1e10) underflows in f32),
so the host keeps only unmasked source positions per batch (gather),
pads each batch to a multiple of 128 columns, transposes the kept enc
columns feature-major and pre-casts to bf16. Batches are assigned to
(core, slot) by sorted compacted width, so the SPMD per-slot tile counts
are the max over cores of the k-th widest batch.

The kernel is PE-bound (264 N=512 bf16 matmuls/core ~= 57us at the warm
216ns/MM cadence), so everything else is arranged to keep the PE
streaming from the first possible moment to the last:
 - Opening DMAs are small interleaved chunks (wq/enc per-ec slices) on
   the sync ring so the first real matmul starts ~2.3MB earlier than a
   coarse-grained fill; warmup matmuls on memset data bridge the fill
   and pre-warm the HAM clock.
 - h @ w_dec + b is computed on the HOST (it only needs h and the small
   w_dec); the device gets dec_rows [4, 512] + a one-hot sel matrix and
   broadcasts to all 128 partitions with 4 cheap matmuls spread through
   slot 0's first group (no 528KB hwdec DMA gating the stream).
 - Slot 0 group 0 runs ec-major (matmuls only need per-ec slices, so
   compute starts after 256KB); all later groups are laid out tile-major
   and run j-major so each PSUM bank retires right after its 8th matmul
   and the drain (dec-add -> tanh -> v-reduce) spreads evenly.
 - Epilogue per group: mask-add (DVE) then a single ACT exp with
   accum_out producing per-partition partial sums; the host sums the
   partials and applies the softmax division during the scatter. No
   partition-broadcast matmul / reciprocal / scale on the critical tail.
 - The last slot ends with a 1-tile group whose dec-add is folded into
   the PE accumulation and whose tanh/v-reduce run in two pipelined
   halves, so the post-matmul tail is short.
"""

import numpy as np
from contextlib import ExitStack

import concourse.bass as bass
import concourse.tile as tile
from concourse import bacc, mybir
from concourse.bass_utils import run_bass_kernel_spmd

# Problem shape (hardcoded; kernel.py must be self-contained).
B, S, E2, DH = 32, 2048, 1024, 512
N_CORES = 8
BC = B // N_CORES        # batches per core = 4
P = 128                  # SBUF partitions
EC = E2 // P             # enc-feature chunks = 8
D = DH                   # 512

f32 = mybir.dt.float32
bf16 = mybir.dt.bfloat16
fp16 = mybir.dt.float16
AF = mybir.ActivationFunctionType

NEG_BIG = -1.0e10
# consts_bf column layout: [v_sb | dec_rows | sel]
CV0 = 0          # v broadcast  [128, D]
CD0 = D          # dec_rows     [4, D]   (partitions 0-3)
CS0 = 2 * D      # sel one-hot  [4, BC*P] (partitions 0-3)
CW = 2 * D + BC * P

_NC_CACHE = {}


def _group_sizes(nt):
    sizes = [4] * (nt // 4)
    if nt % 4:
        sizes.append(nt % 4)
    return sizes


def _slot_group_sizes(widths, b):
    """PSUM-group sizes for slot b. The LAST slot ends with a 1-tile group
    so only one drain chain runs after the kernel's final matmul."""
    w = widths[b]
    if b == len(widths) - 1 and w > 1:
        return _group_sizes(w - 1) + [1]
    return _group_sizes(w)


def _plan(widths):
    """Static layout plan shared by host packing and kernel emission."""
    nslots = len(widths)
    slot_sizes = [_slot_group_sizes(widths, b) for b in range(nslots)]
    coff = {}   # column offset of (slot, group) in enc layout
    c = 0
    for b in range(nslots):
        for sg, gsz in enumerate(slot_sizes[b]):
            coff[(b, sg)] = c
            c += EC * gsz * P
    # output layout: per slot nt exp columns + one partial-sum column per group
    ooff = []
    o = 0
    for b in range(nslots):
        ooff.append(o)
        o += widths[b] + len(slot_sizes[b])
    toff = [sum(widths[:i]) for i in range(nslots)]
    return slot_sizes, coff, ooff, o, toff


def _emit(ctx, tc, nc, widths, enc_t, wq_in, consts_in, madd_in, out):
    nslots = len(widths)
    slot_sizes, coff, ooff, _, toff = _plan(widths)
    ngrp = sum(len(s) for s in slot_sizes)

    const = ctx.enter_context(tc.tile_pool(name="const", bufs=1))
    spsum = ctx.enter_context(tc.tile_pool(name="spsum", bufs=2, space="PSUM"))
    mpsum = ctx.enter_context(tc.tile_pool(name="mpsum", bufs=6, space="PSUM"))
    encp = ctx.enter_context(tc.tile_pool(name="encp", bufs=ngrp))
    tmpp = ctx.enter_context(tc.tile_pool(name="tmpp", bufs=3))
    thp = ctx.enter_context(tc.tile_pool(name="thp", bufs=4))
    scrp = ctx.enter_context(tc.tile_pool(name="scrp", bufs=2))
    epip = ctx.enter_context(tc.tile_pool(name="epip", bufs=2))

    # ---- warmup source tiles (no DMA deps): keep the PE busy during fill ----
    ones16 = const.tile([P, P], fp16)
    nc.vector.memset(ones16[:], 1.0)
    wsrc = const.tile([P, D], fp16)
    nc.vector.memset(wsrc[:], 0.001)

    # ---- SBUF tiles fed by DMA ----
    gtiles = {}
    for b in range(nslots):
        for sg, gsz in enumerate(slot_sizes[b]):
            gtiles[(b, sg)] = encp.tile(
                [P, EC * gsz * P], bf16, tag="enc", name=f"enc_{b}_{sg}"
            )
    wq = const.tile([P, EC * D], bf16)
    csb = const.tile([P, CW], bf16)
    madd_sb = const.tile([P, sum(widths)], f32)
    dec_bc = const.tile([P, nslots * D], f32)

    # ---- DMA: interleaved fine-grained head, then group stream ----
    g00 = gtiles[(0, 0)]
    Wg0 = slot_sizes[0][0] * P
    # sync ring: first matmuls need only wq ec0 + g00 ec0 (ec-major group 0)
    nc.sync.dma_start(out=wq[:, 0:D], in_=wq_in[:, 0:D])
    nc.sync.dma_start(out=g00[:, 0:Wg0], in_=enc_t[:, 0:Wg0])
    nc.sync.dma_start(out=wq[:, D : 4 * D], in_=wq_in[:, D : 4 * D])
    nc.sync.dma_start(out=g00[:, Wg0 : 3 * Wg0], in_=enc_t[:, Wg0 : 3 * Wg0])
    nc.sync.dma_start(out=g00[:, 3 * Wg0 : 8 * Wg0], in_=enc_t[:, 3 * Wg0 : 8 * Wg0])
    nc.sync.dma_start(out=wq[:, 4 * D : 8 * D], in_=wq_in[:, 4 * D : 8 * D])
    # scalar ring: small consts
    nc.scalar.dma_start(out=csb[:], in_=consts_in[:])
    nc.scalar.dma_start(out=madd_sb[:], in_=madd_in[:])
    # remaining groups on sync ring in consumption order; split 4-tile
    # groups at a tile boundary for smoother pipelining
    rest = [(b, sg) for b in range(nslots) for sg in range(len(slot_sizes[b]))][1:]
    for b, sg in rest:
        gsz = slot_sizes[b][sg]
        lo = coff[(b, sg)]
        gt = gtiles[(b, sg)]
        w = EC * gsz * P
        if gsz >= 3:
            h = (gsz // 2) * EC * P
            nc.sync.dma_start(out=gt[:, 0:h], in_=enc_t[:, lo : lo + h])
            nc.sync.dma_start(out=gt[:, h:w], in_=enc_t[:, lo + h : lo + w])
        else:
            nc.sync.dma_start(out=gt[:], in_=enc_t[:, lo : lo + w])

    # ---- PE warmup: matmuls on memset data bridge the DMA fill ----
    wps = spsum.tile([P, D], f32, tag="sp", name="warm")
    for _ in range(5):
        nc.tensor.matmul(wps[:], lhsT=ones16[:], rhs=wsrc[:], start=True, stop=True)

    v_sb = csb[:, CV0 : CV0 + D]

    def emit_dec_mm(b):
        ps = spsum.tile([P, D], f32, tag="sp", name=f"decb_{b}")
        nc.tensor.matmul(
            ps[:],
            lhsT=csb[0:BC, CS0 + b * P : CS0 + (b + 1) * P],
            rhs=csb[0:BC, CD0 : CD0 + D],
            start=True,
            stop=True,
        )
        nc.vector.tensor_copy(dec_bc[:, b * D : (b + 1) * D], ps[:])

    def drain(b, st, ps, att, fold):
        """dec-add -> tanh -> v-reduce for one s-tile; att[:, st] gets the
        logits. fold=True means dec came in via PE accumulation (tanh reads
        PSUM directly, split in two pipelined halves for a short tail)."""
        if fold:
            th = thp.tile([P, D], bf16, tag="th")
            h0 = D // 2
            nc.scalar.activation(th[:, 0:h0], ps[:, 0:h0], AF.Tanh)
            scr = scrp.tile([P, D], bf16, tag="scr")
            a0 = tmpp.tile([P, 1], f32, tag="acc", bufs=2)
            nc.vector.affine_mul_reduce(
                out=scr[:, 0:h0], accum_out=a0[:],
                in0=th[:, 0:h0], in1=v_sb[:, 0:h0], scale=1.0, bias=0.0,
            )
            nc.scalar.activation(th[:, h0:D], ps[:, h0:D], AF.Tanh)
            a1 = tmpp.tile([P, 1], f32, tag="acc", bufs=2)
            nc.vector.affine_mul_reduce(
                out=scr[:, h0:D], accum_out=a1[:],
                in0=th[:, h0:D], in1=v_sb[:, h0:D], scale=1.0, bias=0.0,
            )
            nc.vector.tensor_add(att[:, st : st + 1], a0[:], a1[:])
        else:
            t_sb = tmpp.tile([P, D], f32, tag="tmp")
            nc.vector.tensor_add(t_sb[:], ps[:], dec_bc[:, b * D : (b + 1) * D])
            th = thp.tile([P, D], bf16, tag="th")
            nc.scalar.activation(th[:], t_sb[:], AF.Tanh)
            scr = scrp.tile([P, D], bf16, tag="scr")
            nc.vector.affine_mul_reduce(
                out=scr[:], accum_out=att[:, st : st + 1],
                in0=th[:], in1=v_sb[:], scale=1.0, bias=0.0,
            )

    # ---- main loop over slots ----
    for b in range(nslots):
        nt = widths[b]
        sizes = slot_sizes[b]
        ng = len(sizes)
        starts = [sum(sizes[:i]) for i in range(ng)]
        att = epip.tile([P, nt], f32, tag="att", name=f"att_{b}")
        out_sb = epip.tile([P, nt + ng], f32, tag="osb", name=f"osb_{b}")
        for sg, gsz in enumerate(sizes):
            gt = gtiles[(b, sg)]
            last_group = b == nslots - 1 and sg == ng - 1
            if b == 0 and sg == 0:
                # ec-major: start computing after the first per-ec slices
                # land; dec-broadcast matmuls spread through the middle.
                Wg = gsz * P
                psums = [
                    mpsum.tile([P, D], f32, tag="mm", name=f"mm0_{j}")
                    for j in range(gsz)
                ]
                for ec in range(EC):
                    for j in range(gsz):
                        nc.tensor.matmul(
                            psums[j][:],
                            lhsT=gt[:, ec * Wg + j * P : ec * Wg + (j + 1) * P],
                            rhs=wq[:, ec * D : (ec + 1) * D],
                            start=(ec == 0),
                            stop=(ec == EC - 1),
                        )
                    if 2 <= ec < 2 + nslots:
                        emit_dec_mm(ec - 2)
                for j in range(gsz):
                    drain(b, starts[sg] + j, psums[j], att, fold=False)
            else:
                # tile-major layout, j-major loop: each bank retires right
                # after its own 8 matmuls.
                for j in range(gsz):
                    fold = last_group and j == gsz - 1
                    ps = mpsum.tile([P, D], f32, tag="mm", name=f"mm_{b}_{sg}_{j}")
                    for ec in range(EC):
                        nc.tensor.matmul(
                            ps[:],
                            lhsT=gt[:, (j * EC + ec) * P : (j * EC + ec + 1) * P],
                            rhs=wq[:, ec * D : (ec + 1) * D],
                            start=(ec == 0),
                            stop=(ec == EC - 1) and not fold,
                        )
                    if fold:
                        nc.tensor.matmul(
                            ps[:],
                            lhsT=csb[0:BC, CS0 + b * P : CS0 + (b + 1) * P],
                            rhs=csb[0:BC, CD0 : CD0 + D],
                            start=False,
                            stop=True,
                        )
                    drain(b, starts[sg] + j, ps, att, fold=fold)
            # group epilogue: mask-add then exp with per-partition accum
            g0, g1 = starts[sg], starts[sg] + gsz
            attm = epip.tile([P, gsz], f32, tag="attm", name=f"attm_{b}_{sg}")
            nc.vector.tensor_add(
                attm[:], att[:, g0:g1], madd_sb[:, toff[b] + g0 : toff[b] + g1]
            )
            nc.scalar.activation(
                out_sb[:, g0:g1], attm[:], AF.Exp,
                accum_out=out_sb[:, nt + sg : nt + sg + 1],
            )
        nc.sync.dma_start(
            out=out[:, ooff[b] : ooff[b] + nt + ng], in_=out_sb[:]
        )


def build_nc(widths):
    key = tuple(widths)
    if key in _NC_CACHE:
        return _NC_CACHE[key]
    ntot = sum(widths)
    _, _, _, osz, _ = _plan(widths)
    nc = bacc.Bacc("TRN2", target_bir_lowering=False, debug=False)
    enc_t = nc.dram_tensor(
        "enc_t", [P, EC * P * ntot], bf16, kind="ExternalInput"
    ).ap()
    wq_in = nc.dram_tensor("wq_in", [P, EC * D], bf16, kind="ExternalInput").ap()
    consts = nc.dram_tensor("consts", [P, CW], bf16, kind="ExternalInput").ap()
    madd = nc.dram_tensor("madd", [P, ntot], f32, kind="ExternalInput").ap()
    out = nc.dram_tensor("out", [P, osz], f32, kind="ExternalOutput").ap()

    with tile.TileContext(nc) as tc:
        with ExitStack() as ctx:
            _emit(ctx, tc, nc, list(widths), enc_t, wq_in, consts, madd, out)
    nc.compile()
    _NC_CACHE[key] = nc
    return nc


def plan_assignment(counts):
    """Sort batches by compacted tile count; rank k -> core k%8, slot k//8.
    Returns (assign[core][slot] = global batch, widths[slot])."""
    tiles = np.maximum(1, np.ceil(counts / P).astype(int))
    order = sorted(range(B), key=lambda gb: (-tiles[gb], -counts[gb], gb))
    assign = [[-1] * BC for _ in range(N_CORES)]
    widths = []
    for slot in range(BC):
        ranks = order[slot * N_CORES : (slot + 1) * N_CORES]
        for c, gb in enumerate(ranks):
            assign[c][slot] = gb
        widths.append(max(int(tiles[gb]) for gb in ranks))
    return assign, widths


def shard_inputs(inputs, assign, widths):
    import ml_dtypes

    h = np.asarray(inputs["h"], dtype=np.float32)
    enc = np.asarray(inputs["enc_output"], dtype=np.float32)
    mask = np.asarray(inputs["mask"], dtype=np.int32)
    attn_w = np.asarray(inputs["attn_w"], dtype=np.float32)
    attn_b = np.asarray(inputs["attn_b"], dtype=np.float32)
    v_w = np.asarray(inputs["v_w"], dtype=np.float32)

    nslots = len(widths)
    ntot = sum(widths)
    slot_sizes, _, _, _, toff = _plan(widths)

    w_dec = attn_w[:DH]                   # [DH, D]
    # w_enc [E2, D] -> [P, EC*D] with free index (ec, d), pre-cast to bf16
    wq = np.ascontiguousarray(
        attn_w[DH:].reshape(EC, P, D).transpose(1, 0, 2).reshape(P, EC * D)
    ).astype(ml_dtypes.bfloat16)

    kept = [np.nonzero(mask[gb])[0] for gb in range(B)]

    in_maps = []
    for c in range(N_CORES):
        perm = assign[c]
        enc_c = np.zeros((P, EC * P * ntot), dtype=ml_dtypes.bfloat16)
        madd = np.zeros((P, ntot), dtype=np.float32)
        consts = np.zeros((P, CW), dtype=ml_dtypes.bfloat16)
        consts[:, CV0 : CV0 + D] = v_w.astype(ml_dtypes.bfloat16)[None, :]
        dec_rows = (h[perm] @ w_dec + attn_b).astype(ml_dtypes.bfloat16)
        consts[0:BC, CD0 : CD0 + D] = dec_rows
        for b in range(nslots):
            consts[b, CS0 + b * P : CS0 + (b + 1) * P] = 1.0

        col = 0
        for b in range(nslots):
            gb = perm[b]
            W = widths[b] * P
            idx = kept[gb]
            n = len(idx)
            # kept enc columns, feature-major, padded: [EC, P, W]
            padded = np.zeros((EC, P, W), dtype=ml_dtypes.bfloat16)
            cols = enc[idx, gb, :].T.astype(ml_dtypes.bfloat16)
            padded[:, :, :n] = cols.reshape(EC, P, n)
            off = 0
            for sg, gsz in enumerate(slot_sizes[b]):
                blk = padded[:, :, off : off + gsz * P]      # [EC, P, Wg]
                w = EC * gsz * P
                if b == 0 and sg == 0:
                    # ec-major: cols (ec, j, p)
                    enc_c[:, col : col + w] = blk.transpose(1, 0, 2).reshape(P, w)
                else:
                    # tile-major: cols (j, ec, p)
                    enc_c[:, col : col + w] = (
                        blk.reshape(EC, P, gsz, P)
                        .transpose(1, 2, 0, 3)
                        .reshape(P, w)
                    )
                off += gsz * P
                col += w
            # additive mask: 0 for real columns, -1e10 for pads
            m = np.zeros(W, dtype=np.float32)
            m[n:] = NEG_BIG
            madd[:, toff[b] : toff[b] + widths[b]] = m.reshape(widths[b], P).T
        in_maps.append(dict(enc_t=enc_c, wq_in=wq, consts=consts, madd=madd))
    return in_maps, kept


def run(inputs, trace=False):
    mask = np.asarray(inputs["mask"], dtype=np.int32)
    counts = mask.sum(axis=1)
    assign, widths = plan_assignment(counts)
    nc = build_nc(widths)
    in_maps, kept = shard_inputs(inputs, assign, widths)
    res = run_bass_kernel_spmd(nc, in_maps, list(range(N_CORES)), trace=trace)
    slot_sizes, _, ooff, osz, _ = _plan(widths)
    out_full = np.zeros((B, S), dtype=np.float32)
    for c in range(N_CORES):
        vals = res.results[c]["out"].reshape(P, osz)
        for b in range(len(widths)):
            gb = assign[c][b]
            idx = kept[gb]
            nt = widths[b]
            ng = len(slot_sizes[b])
            region = vals[:, ooff[b] : ooff[b] + nt + ng]
            denom = region[:, nt:].sum(dtype=np.float32)
            flat = region[:, :nt].T.reshape(nt * P)
            out_full[gb, idx] = flat[: len(idx)] / denom
    return out_full, res


def kernel(**inputs) -> np.ndarray:
    out, _ = run(inputs, trace=False)
    return out


# revision 14
# speedup vs baseline: 1.0605x; 1.0132x over previous
"""Bass/Trainium2 kernel for nn_Attention_84688165142614 (additive attention).

Computes, for full inputs (B=32, S=2048, EH=512, DH=512):
    enc    = enc_output.transpose(1, 0, 2)                  # [B, S, 2EH]
    energy = tanh(enc @ w_enc + (h @ w_dec) + attn_b)       # [B, S, DH]
    att    = energy @ v_w                                   # [B, S]
    att    = where(mask == 0, -1e10, att)
    out    = softmax(att, axis=1)

Strategy: data-parallel over batch across 8 NeuronCores (4 batches/core),
plus mask-sparsity compaction: the host keeps only unmasked source
positions per batch (their reference softmax output is exactly 0), pads
each batch to a multiple of 128 columns, transposes the kept enc columns
feature-major and pre-casts to bf16. Batches are assigned to (core, slot)
by sorted compacted width so the SPMD per-slot tile counts are the max
over cores of the k-th widest batch.

The kernel is PE-bound (~272 N=512 bf16 matmuls/core ~= 59us at the warm
216ns/MM cadence), so everything else is arranged to keep the PE
streaming from the first possible moment to the last, and to keep the
vector engine (the co-bottleneck: dec-add + v-reduce per tile) under the
PE's rate:
 - Head transfers (wq / first enc group, per-ec slices) are separate
   CONTIGUOUS DRAM tensors: a column-slice of the big enc tensor has
   1KB rows strided 68KB apart in HBM and moves at ~30GB/s, while the
   same bytes contiguous move at ~300GB/s. First real matmul ~9.5us.
 - 6 warmup matmuls on memset data bridge the fill with zero PE gaps so
   the HAM clock gate opens (1.2 -> 2.4 GHz) ~3.4us after the stream
   starts and never re-throttles.
 - h @ w_dec + b is computed on the HOST; the device broadcasts it to
   all 128 partitions with 4 cheap one-hot matmuls spread through slot
   0's first group, drained by ACT-engine copies (vector stays free).
 - Slot 0 group 0 runs ec-major (compute starts after one 128KB slice);
   later groups are laid out tile-major and run j-major so each PSUM
   bank retires right after its own 8 matmuls and drains spread evenly.
 - Per-tile drain: DVE add (PSUM + dec broadcast), ACT tanh, then a
   native tensor_tensor_reduce (mul + add-reduce) for att = energy @ v.
 - Epilogue per group: exp with accum_out partial sums (single-tile
   groups fold the pad mask into the exp bias); the host sums partials
   and applies the softmax division during the scatter. The last slot
   ends with a 1-tile group whose dec-add is folded into the PE
   accumulation, and its output DMA is split so the final transfer after
   the last exp is a few hundred bytes.
"""

import numpy as np
from contextlib import ExitStack

import concourse.bass as bass
import concourse.tile as tile
from concourse import bacc, mybir
from concourse.bass_utils import run_bass_kernel_spmd

# Problem shape (hardcoded; kernel.py must be self-contained).
B, S, E2, DH = 32, 2048, 1024, 512
N_CORES = 8
BC = B // N_CORES        # batches per core = 4
P = 128                  # SBUF partitions
EC = E2 // P             # enc-feature chunks = 8
D = DH                   # 512

f32 = mybir.dt.float32
bf16 = mybir.dt.bfloat16
fp16 = mybir.dt.float16
AF = mybir.ActivationFunctionType
ALU = mybir.AluOpType

NEG_BIG = -1.0e10
# consts column layout: [v_sb | dec_rows | sel]
CV0 = 0          # v broadcast  [128, D]
CD0 = D          # dec_rows     [4, D]   (partitions 0-3)
CS0 = 2 * D      # sel one-hot  [4, BC*P] (partitions 0-3)
CW = 2 * D + BC * P

N_WARMUP = 6
# HW bisect flags
USE_ACT_COPY = True      # dec_bc drain on ACT (vs vector copy)
USE_EXP_BIAS = False     # 1-col groups: pad mask via exp bias (vs add+exp)
USE_SPLIT_OUT = False    # last slot: split out DMA
VRED = "stt"             # v-reduce impl: "stt" | "ttr" | "amr"

_NC_CACHE = {}


def _group_sizes(nt):
    sizes = [4] * (nt // 4)
    if nt % 4:
        sizes.append(nt % 4)
    return sizes


def _slot_group_sizes(widths, b):
    """PSUM-group sizes for slot b. The LAST slot ends with a 1-tile group
    so only one drain chain runs after the kernel's final matmul."""
    w = widths[b]
    if b == len(widths) - 1 and w > 1:
        return _group_sizes(w - 1) + [1]
    return _group_sizes(w)


def _plan(widths):
    """Static layout plan shared by host packing and kernel emission.

    Output region per slot: [nt exp columns | one partial-sum column per
    group of >= 2 tiles]. Single-tile groups' partials are just their exp
    column; the host adds those directly.
    """
    nslots = len(widths)
    slot_sizes = [_slot_group_sizes(widths, b) for b in range(nslots)]
    ooff = []
    o = 0
    for b in range(nslots):
        ooff.append(o)
        o += widths[b] + sum(1 for g in slot_sizes[b] if g >= 2)
    toff = [sum(widths[:i]) for i in range(nslots)]
    return slot_sizes, ooff, o, toff


def _head_splits(slot_sizes):
    """The first group's per-ec DRAM tensors: ec0 alone, ec1-2, ec3-7."""
    Wg0 = slot_sizes[0][0] * P
    return [("e00a", 0, Wg0), ("e00b", Wg0, 3 * Wg0), ("e00c", 3 * Wg0, 8 * Wg0)]


def _emit(ctx, tc, nc, widths, tens):
    nslots = len(widths)
    slot_sizes, ooff, _, toff = _plan(widths)
    ngrp = sum(len(s) for s in slot_sizes)
    out = tens["out"]

    const = ctx.enter_context(tc.tile_pool(name="const", bufs=1))
    spsum = ctx.enter_context(tc.tile_pool(name="spsum", bufs=2, space="PSUM"))
    mpsum = ctx.enter_context(tc.tile_pool(name="mpsum", bufs=6, space="PSUM"))
    encp = ctx.enter_context(tc.tile_pool(name="encp", bufs=ngrp))
    tmpp = ctx.enter_context(tc.tile_pool(name="tmpp", bufs=3))
    thp = ctx.enter_context(tc.tile_pool(name="thp", bufs=4))
    scrp = ctx.enter_context(tc.tile_pool(name="scrp", bufs=2))
    epip = ctx.enter_context(tc.tile_pool(name="epip", bufs=2))

    # ---- warmup source tiles (no DMA deps): keep the PE busy during fill ----
    ones16 = const.tile([P, P], fp16)
    nc.vector.memset(ones16[:], 1.0)
    wsrc = const.tile([P, D], fp16)
    nc.vector.memset(wsrc[:], 0.001)

    # ---- SBUF tiles fed by DMA ----
    gtiles = {}
    for b in range(nslots):
        for sg, gsz in enumerate(slot_sizes[b]):
            gtiles[(b, sg)] = encp.tile(
                [P, EC * gsz * P], bf16, tag="enc", name=f"enc_{b}_{sg}"
            )
    wq = const.tile([P, EC * D], bf16)
    csb = const.tile([P, CW], bf16)
    madd_sb = const.tile([P, sum(widths)], f32)
    dec_bc = const.tile([P, nslots * D], f32)

    # ---- DMA: contiguous fine-grained head, then the group stream ----
    g00 = gtiles[(0, 0)]
    Wg0 = slot_sizes[0][0] * P
    nc.sync.dma_start(out=wq[:, 0:D], in_=tens["wq_a"][:])
    nc.sync.dma_start(out=g00[:, 0:Wg0], in_=tens["e00a"][:])
    nc.sync.dma_start(out=wq[:, D : 4 * D], in_=tens["wq_b"][:])
    nc.sync.dma_start(out=g00[:, Wg0 : 3 * Wg0], in_=tens["e00b"][:])
    nc.sync.dma_start(out=g00[:, 3 * Wg0 : 8 * Wg0], in_=tens["e00c"][:])
    nc.sync.dma_start(out=wq[:, 4 * D : 8 * D], in_=tens["wq_c"][:])
    # scalar ring: small consts
    nc.scalar.dma_start(out=csb[:], in_=tens["consts"][:])
    nc.scalar.dma_start(out=madd_sb[:], in_=tens["madd"][:])
    # remaining groups: single transfers from one big tensor, in
    # consumption order (1MB strided transfers run at ~340GB/s)
    enc_t = tens["enc_t"]
    rest = [(b, sg) for b in range(nslots) for sg in range(len(slot_sizes[b]))][1:]
    roff = 0
    for b, sg in rest:
        gsz = slot_sizes[b][sg]
        w = EC * gsz * P
        nc.sync.dma_start(out=gtiles[(b, sg)][:], in_=enc_t[:, roff : roff + w])
        roff += w

    # ---- PE warmup: matmuls on memset data bridge the DMA fill ----
    wps = spsum.tile([P, D], f32, tag="sp", name="warm")
    for _ in range(N_WARMUP):
        nc.tensor.matmul(wps[:], lhsT=ones16[:], rhs=wsrc[:], start=True, stop=True)

    v_sb = csb[:, CV0 : CV0 + D]

    def emit_dec_mm(b):
        ps = spsum.tile([P, D], f32, tag="sp", name=f"decb_{b}")
        nc.tensor.matmul(
            ps[:],
            lhsT=csb[0:BC, CS0 + b * P : CS0 + (b + 1) * P],
            rhs=csb[0:BC, CD0 : CD0 + D],
            start=True,
            stop=True,
        )
        # drain on ACT: the vector engine is the co-bottleneck
        if USE_ACT_COPY:
            nc.scalar.activation(dec_bc[:, b * D : (b + 1) * D], ps[:], AF.Copy)
        else:
            nc.vector.tensor_copy(dec_bc[:, b * D : (b + 1) * D], ps[:])

    def drain(b, st, ps, att, fold):
        """dec-add -> tanh -> v-reduce for one s-tile; att[:, st] gets the
        logits. fold=True means dec came in via PE accumulation (tanh reads
        PSUM directly)."""
        th = thp.tile([P, D], bf16, tag="th")
        if fold:
            nc.scalar.activation(th[:], ps[:], AF.Tanh)
        else:
            t_sb = tmpp.tile([P, D], f32, tag="tmp")
            nc.vector.tensor_add(t_sb[:], ps[:], dec_bc[:, b * D : (b + 1) * D])
            nc.scalar.activation(th[:], t_sb[:], AF.Tanh)
        scr = scrp.tile([P, D], bf16, tag="scr")
        if VRED == "stt":
            nc.vector.scalar_tensor_tensor(
                out=scr[:],
                in0=th[:],
                scalar=1.0,
                in1=v_sb,
                op0=ALU.bypass,
                op1=ALU.mult,
                accum_out=att[:, st : st + 1],
            )
        elif VRED == "ttr":
            nc.vector.tensor_tensor_reduce(
                out=scr[:],
                in0=th[:],
                in1=v_sb,
                scale=1.0,
                scalar=0.0,
                op0=ALU.mult,
                op1=ALU.add,
                accum_out=att[:, st : st + 1],
            )
        else:
            nc.vector.affine_mul_reduce(
                out=scr[:], accum_out=att[:, st : st + 1],
                in0=th[:], in1=v_sb, scale=1.0, bias=0.0,
            )

    # ---- main loop over slots ----
    for b in range(nslots):
        nt = widths[b]
        sizes = slot_sizes[b]
        ng = len(sizes)
        npart = sum(1 for g in sizes if g >= 2)
        starts = [sum(sizes[:i]) for i in range(ng)]
        last_slot = b == nslots - 1
        att = epip.tile([P, nt], f32, tag="att", name=f"att_{b}")
        out_sb = epip.tile([P, nt + npart], f32, tag="osb", name=f"osb_{b}")
        part_i = 0
        for sg, gsz in enumerate(sizes):
            gt = gtiles[(b, sg)]
            fold_group = last_slot and sg == ng - 1
            if b == 0 and sg == 0:
                # ec-major: start computing after the first per-ec slices
                # land; dec-broadcast matmuls spread through the middle.
                Wg = gsz * P
                psums = [
                    mpsum.tile([P, D], f32, tag="mm", name=f"mm0_{j}")
                    for j in range(gsz)
                ]
                for ec in range(EC):
                    for j in range(gsz):
                        nc.tensor.matmul(
                            psums[j][:],
                            lhsT=gt[:, ec * Wg + j * P : ec * Wg + (j + 1) * P],
                            rhs=wq[:, ec * D : (ec + 1) * D],
                            start=(ec == 0),
                            stop=(ec == EC - 1),
                        )
                    if 2 <= ec < 2 + nslots:
                        emit_dec_mm(ec - 2)
                for j in range(gsz):
                    drain(b, starts[sg] + j, psums[j], att, fold=False)
            else:
                # tile-major layout, j-major loop: each bank retires right
                # after its own 8 matmuls.
                for j in range(gsz):
                    fold = fold_group and j == gsz - 1
                    ps = mpsum.tile([P, D], f32, tag="mm", name=f"mm_{b}_{sg}_{j}")
                    for ec in range(EC):
                        nc.tensor.matmul(
                            ps[:],
                            lhsT=gt[:, (j * EC + ec) * P : (j * EC + ec + 1) * P],
                            rhs=wq[:, ec * D : (ec + 1) * D],
                            start=(ec == 0),
                            stop=(ec == EC - 1) and not fold,
                        )
                    if fold:
                        nc.tensor.matmul(
                            ps[:],
                            lhsT=csb[0:BC, CS0 + b * P : CS0 + (b + 1) * P],
                            rhs=csb[0:BC, CD0 : CD0 + D],
                            start=False,
                            stop=True,
                        )
                    drain(b, starts[sg] + j, ps, att, fold=fold)
            # group epilogue: exp with per-partition accum. Single-tile
            # groups fold the pad mask into the exp bias (no partial col —
            # the exp column IS the partial).
            g0, g1 = starts[sg], starts[sg] + gsz
            if gsz == 1 and USE_EXP_BIAS:
                nc.scalar.activation(
                    out_sb[:, g0:g1], att[:, g0:g1], AF.Exp,
                    bias=madd_sb[:, toff[b] + g0 : toff[b] + g1],
                )
            elif gsz == 1:
                attm = epip.tile([P, gsz], f32, tag="attm", name=f"attm_{b}_{sg}")
                nc.vector.tensor_add(
                    attm[:], att[:, g0:g1], madd_sb[:, toff[b] + g0 : toff[b] + g1]
                )
                nc.scalar.activation(out_sb[:, g0:g1], attm[:], AF.Exp)
            else:
                attm = epip.tile([P, gsz], f32, tag="attm", name=f"attm_{b}_{sg}")
                nc.vector.tensor_add(
                    attm[:], att[:, g0:g1], madd_sb[:, toff[b] + g0 : toff[b] + g1]
                )
                nc.scalar.activation(
                    out_sb[:, g0:g1], attm[:], AF.Exp,
                    accum_out=out_sb[:, nt + part_i : nt + part_i + 1],
                )
                part_i += 1
        lo, hi = ooff[b], ooff[b] + nt + npart
        if last_slot and nt >= 2 and USE_SPLIT_OUT:
            # everything but the last exp column goes out early; the final
            # transfer after the last exp is a few hundred bytes
            nc.sync.dma_start(out=out[:, lo : lo + nt - 1], in_=out_sb[:, 0 : nt - 1])
            nc.sync.dma_start(out=out[:, lo + nt - 1 : hi], in_=out_sb[:, nt - 1 :])
        else:
            nc.sync.dma_start(out=out[:, lo:hi], in_=out_sb[:])


def build_nc(widths):
    key = tuple(widths)
    if key in _NC_CACHE:
        return _NC_CACHE[key]
    slot_sizes, _, osz, _ = _plan(widths)
    nc = bacc.Bacc("TRN2", target_bir_lowering=False, debug=False)
    tens = {}

    def inp(name, shape, dtype=bf16):
        tens[name] = nc.dram_tensor(name, shape, dtype, kind="ExternalInput").ap()

    inp("wq_a", [P, D])
    inp("wq_b", [P, 3 * D])
    inp("wq_c", [P, 4 * D])
    inp("consts", [P, CW])
    inp("madd", [P, sum(widths)], f32)
    for name, lo, hi in _head_splits(slot_sizes):
        inp(name, [P, hi - lo])
    rest_w = EC * P * (sum(widths) - slot_sizes[0][0])
    inp("enc_t", [P, rest_w])
    tens["out"] = nc.dram_tensor("out", [P, osz], f32, kind="ExternalOutput").ap()

    with tile.TileContext(nc) as tc:
        with ExitStack() as ctx:
            _emit(ctx, tc, nc, list(widths), tens)
    nc.compile()
    _NC_CACHE[key] = nc
    return nc


def plan_assignment(counts):
    """Sort batches by compacted tile count; rank k -> core k%8, slot k//8.
    Returns (assign[core][slot] = global batch, widths[slot])."""
    tiles = np.maximum(1, np.ceil(counts / P).astype(int))
    order = sorted(range(B), key=lambda gb: (-tiles[gb], -counts[gb], gb))
    assign = [[-1] * BC for _ in range(N_CORES)]
    widths = []
    for slot in range(BC):
        ranks = order[slot * N_CORES : (slot + 1) * N_CORES]
        for c, gb in enumerate(ranks):
            assign[c][slot] = gb
        widths.append(max(int(tiles[gb]) for gb in ranks))
    return assign, widths


def shard_inputs(inputs, assign, widths):
    import ml_dtypes

    h = np.asarray(inputs["h"], dtype=np.float32)
    enc = np.asarray(inputs["enc_output"], dtype=np.float32)
    mask = np.asarray(inputs["mask"], dtype=np.int32)
    attn_w = np.asarray(inputs["attn_w"], dtype=np.float32)
    attn_b = np.asarray(inputs["attn_b"], dtype=np.float32)
    v_w = np.asarray(inputs["v_w"], dtype=np.float32)

    nslots = len(widths)
    ntot = sum(widths)
    slot_sizes, _, _, toff = _plan(widths)

    w_dec = attn_w[:DH]                   # [DH, D]
    # w_enc [E2, D] -> [P, EC*D] with free index (ec, d), pre-cast to bf16
    wq = np.ascontiguousarray(
        attn_w[DH:].reshape(EC, P, D).transpose(1, 0, 2).reshape(P, EC * D)
    ).astype(ml_dtypes.bfloat16)
    wq_a = np.ascontiguousarray(wq[:, 0:D])
    wq_b = np.ascontiguousarray(wq[:, D : 4 * D])
    wq_c = np.ascontiguousarray(wq[:, 4 * D : 8 * D])

    kept = [np.nonzero(mask[gb])[0] for gb in range(B)]

    in_maps = []
    for c in range(N_CORES):
        perm = assign[c]
        madd = np.zeros((P, ntot), dtype=np.float32)
        consts = np.zeros((P, CW), dtype=ml_dtypes.bfloat16)
        consts[:, CV0 : CV0 + D] = v_w.astype(ml_dtypes.bfloat16)[None, :]
        dec_rows = (h[perm] @ w_dec + attn_b).astype(ml_dtypes.bfloat16)
        consts[0:BC, CD0 : CD0 + D] = dec_rows
        for b in range(nslots):
            consts[b, CS0 + b * P : CS0 + (b + 1) * P] = 1.0

        im = dict(wq_a=wq_a, wq_b=wq_b, wq_c=wq_c, consts=consts)
        rest_w = EC * P * (ntot - slot_sizes[0][0])
        enc_c = np.zeros((P, rest_w), dtype=ml_dtypes.bfloat16)
        col = 0
        for b in range(nslots):
            gb = perm[b]
            W = widths[b] * P
            idx = kept[gb]
            n = len(idx)
            # kept enc columns, feature-major, padded: [EC, P, W]
            padded = np.zeros((EC, P, W), dtype=ml_dtypes.bfloat16)
            cols = enc[idx, gb, :].T.astype(ml_dtypes.bfloat16)
            padded[:, :, :n] = cols.reshape(EC, P, n)
            off = 0
            for sg, gsz in enumerate(slot_sizes[b]):
                blk = padded[:, :, off : off + gsz * P]      # [EC, P, Wg]
                w = EC * gsz * P
                if b == 0 and sg == 0:
                    # ec-major: cols (ec, j, p); three contiguous head tensors
                    flat = blk.transpose(1, 0, 2).reshape(P, w)
                    for name, lo, hi in _head_splits(slot_sizes):
                        im[name] = np.ascontiguousarray(flat[:, lo:hi])
                else:
                    # tile-major: cols (j, ec, p)
                    enc_c[:, col : col + w] = (
                        blk.reshape(EC, P, gsz, P)
                        .transpose(1, 2, 0, 3)
                        .reshape(P, w)
                    )
                    col += w
                off += gsz * P
            # additive mask: 0 for real columns, -1e10 for pads
            m = np.zeros(W, dtype=np.float32)
            m[n:] = NEG_BIG
            madd[:, toff[b] : toff[b] + widths[b]] = m.reshape(widths[b], P).T
        im["madd"] = madd
        im["enc_t"] = enc_c
        in_maps.append(im)
    return in_maps, kept


def run(inputs, trace=False):
    mask = np.asarray(inputs["mask"], dtype=np.int32)
    counts = mask.sum(axis=1)
    assign, widths = plan_assignment(counts)
    nc = build_nc(widths)
    in_maps, kept = shard_inputs(inputs, assign, widths)
    res = run_bass_kernel_spmd(nc, in_maps, list(range(N_CORES)), trace=trace)
    slot_sizes, ooff, osz, _ = _plan(widths)
    out_full = np.zeros((B, S), dtype=np.float32)
    for c in range(N_CORES):
        vals = res.results[c]["out"].reshape(P, osz)
        for b in range(len(widths)):
            gb = assign[c][b]
            idx = kept[gb]
            nt = widths[b]
            sizes = slot_sizes[b]
            npart = sum(1 for g in sizes if g >= 2)
            region = vals[:, ooff[b] : ooff[b] + nt + npart]
            denom = region[:, nt:].sum(dtype=np.float32)
            st = 0
            for gsz in sizes:
                if gsz == 1:
                    denom += region[:, st].sum(dtype=np.float32)
                st += gsz
            flat = region[:, :nt].T.reshape(nt * P)
            out_full[gb, idx] = flat[: len(idx)] / denom
    return out_full, res


def kernel(**inputs) -> np.ndarray:
    out, _ = run(inputs, trace=False)
    return out
